# revision 14
# baseline (speedup 1.0000x reference)
"""ChatGLM2 attention on 8 TRN2 NeuronCores.

Sharding (SPMD, identical graph on all 8 cores; all per-core variation is in
host-prepared input values):
  core c = h*4 + t
    h in {0,1}: head-half = KV group (16 q-heads each)
    t in {0..3}: query-token block: batch b = t//2, seq half t%2 (1024 tokens)
  Each core computes fused-QKV for its columns, RoPE, attention for its
  (16 heads x 1024 q-tokens x 2048 keys), and the o_proj partial for its
  head-half. Host sums the two head-half partials per token block.

Layout: everything "transposed" (feature-major).  fused^T = W^T.T @ x^T gives
q,k d-major directly; scores are computed as S^T = K @ Q^T tiles
[128 keys x 512 toks]; softmax sums via ones-matmul; P@V via
lhsT = V (keys-major, from 16 PE transposes), rhs = exp(S^T).
The head_dim of q/k (and rope tables) is permuted to [evens, odds] so the
RoPE pair-swap is two contiguous 64-partition copies.  The 1/sqrt(128)
q-scale is folded into the q weights/bias host-side.  All DRAM inputs are
pre-blocked host-side so each device load is one big multi-dim DMA.
"""

import math

import numpy as np
import ml_dtypes

import concourse.bass as bass
import concourse.mybir as mybir
import concourse.tile as tile
from concourse import bacc
from concourse.bass_utils import run_bass_kernel_spmd
from concourse.masks import make_identity

B, S, NS = 2, 2048, 4096
NH, D, NG = 32, 128, 2
NTOK = B * S           # 4096
QTOK = 1024            # q tokens per core
KTOK = 2048            # keys (= batch tokens) per core
NHL = 16               # heads per core
FD = 512               # matmul free dim / psum bank
KB = KTOK // 128       # 16 key blocks
TC = QTOK // FD        # 2 q token chunks
TCK = KTOK // FD       # 4 batch token chunks
NSB = NS // FD         # 8 n_state chunks
NSK = NS // 128        # 32 contraction chunks
NCB = 2 + NHL          # fused col blocks: k, v, q0..q15

F32 = mybir.dt.float32
BF16 = mybir.dt.bfloat16
FP8 = mybir.dt.float8e4
DR = mybir.MatmulPerfMode.DoubleRow
E8SC = 2.0 ** -13      # fp8 exp copy scale (folded back via ones=2^13)

LAST_EXEC_NS = None


def build_nc(with_mask: bool):
    nc = bacc.Bacc(None, target_bir_lowering=False)

    # x blocked [tch, half, kkh, p, f]
    x_all = nc.declare_dram_parameter("x_all", [TCK, 4, NSK // 4, 128, FD],
                                      BF16, isOutput=False)
    # fused weights blocked [colblk(k,v,q0..15), kk, p(ns), c(col)]
    w_all = nc.declare_dram_parameter("w_all", [NCB, NSK, 128, 128],
                                      BF16, isOutput=False)
    b_all = nc.declare_dram_parameter("b_all", [D, NCB], F32, isOutput=False)
    ck = nc.declare_dram_parameter("ck", [D, KTOK], BF16, isOutput=False)
    sk = nc.declare_dram_parameter("sk", [D, KTOK], BF16, isOutput=False)
    # o_w blocked [nb, hd, p(d), f(ns)]
    o_wb = nc.declare_dram_parameter("o_wb", [NSB, NHL, 128, FD],
                                     BF16, isOutput=False)
    if with_mask:
        maskT = nc.declare_dram_parameter("maskT", [KTOK, QTOK], F32,
                                          isOutput=False)

    outp = nc.declare_dram_parameter("outp", [QTOK, NS], F32, isOutput=True)
    k_out = nc.declare_dram_parameter("k_out", [D, KTOK], F32, isOutput=True)
    v_out = nc.declare_dram_parameter("v_out", [D, KTOK], F32, isOutput=True)

    COPY = mybir.ActivationFunctionType.Copy
    EXP = mybir.ActivationFunctionType.Exp

    with tile.TileContext(nc) as tc:
        with (
            tc.tile_pool(name="const", bufs=1) as constp,
            tc.tile_pool(name="persist", bufs=1) as persist,
            tc.tile_pool(name="work", bufs=2) as work,
            tc.tile_pool(name="wq", bufs=2) as wqp,
            tc.tile_pool(name="xrhs", bufs=5) as xrhsp,
            tc.tile_pool(name="owr", bufs=2) as owrp,
            tc.tile_pool(name="maskp", bufs=2) as maskp,
            tc.tile_pool(name="expp", bufs=4) as expp,
            tc.tile_pool(name="outc", bufs=3) as outcp,
            tc.tile_pool(name="pss", bufs=4, space="PSUM") as pss,
            tc.tile_pool(name="pssum", bufs=2, space="PSUM") as pssum,
            tc.tile_pool(name="pspv", bufs=2, space="PSUM") as pspv,
        ):
            # ---- constants ----
            ident = constp.tile([128, 128], F32)
            make_identity(nc, ident)
            ones_mat = constp.tile([128, 128], BF16)
            nc.gpsimd.memset(ones_mat, 1.0)
            b_sb = constp.tile([D, NCB], F32)
            nc.sync.dma_start(b_sb, b_all[:, :])
            ck_sb = constp.tile([D, KTOK], BF16)
            nc.sync.dma_start(ck_sb, ck[:, :])
            sk_sb = constp.tile([D, KTOK], BF16)
            nc.sync.dma_start(sk_sb, sk[:, :])

            # ---- persistent activations ----
            k_bf = persist.tile([128, KTOK], BF16)       # roped k (perm d-major)
            vT_bf = persist.tile([128, KB, 128], BF16)   # v keys-major chunks
            q_heads = [persist.tile([128, QTOK], BF16, tag=f"qh{i}",
                                    name=f"qh{i}")
                       for i in range(NHL)]
            att_bf = persist.tile([128, NHL, QTOK], BF16)

            def rope_chunk(dst_bf, src_f32, toff, n):
                """dst = (src*cos + swap(src)*sin) -> bf16; cols
                toff:toff+n of the rope tables. src [128, n] f32 perm-d."""
                sw = work.tile([128, FD], F32, tag="ropesw")
                nc.vector.tensor_copy(sw[0:64, :n], src_f32[64:128, :])
                nc.vector.tensor_copy(sw[64:128, :n], src_f32[0:64, :])
                t1 = work.tile([128, FD], F32, tag="ropet1")
                nc.vector.tensor_mul(t1[:, :n], src_f32, ck_sb[:, toff:toff + n])
                nc.vector.tensor_mul(sw[:, :n], sw[:, :n], sk_sb[:, toff:toff + n])
                nc.vector.tensor_add(t1[:, :n], t1[:, :n], sw[:, :n])
                nc.vector.tensor_copy(dst_bf, t1[:, :n])
                return t1

            # ---- fused QKV pass, streamed over batch-token chunks ----
            # tch order: kv-only chunks first so k/v finish early.
            for tch in (0, 1, 2, 3):
                xh = []
                for qt in range(4):
                    xt = xrhsp.tile([128, NSK // 4, FD], BF16, tag="xr")
                    nc.sync.dma_start(
                        xt, x_all[tch, qt].rearrange("k p f -> p k f"))
                    xh.append(xt)
                cbs = (list(range(2, NCB)) + [0, 1]) if tch < TC else [0, 1]
                for cb in cbs:
                    wt = wqp.tile([128, NSK, 128], BF16, tag="w")
                    nc.sync.dma_start(
                        wt, w_all[cb].rearrange("k p c -> p k c"))
                    ps = pss.tile([128, FD], F32, tag="s")
                    for kk in range(NSK):
                        nc.tensor.matmul(ps, wt[:, kk, :],
                                         xh[kk // 8][:, kk % 8, :],
                                         start=(kk == 0), stop=(kk == NSK - 1))
                    f_sb = work.tile([128, FD], F32, tag="fsb")
                    nc.vector.tensor_scalar_add(f_sb, ps, b_sb[:, cb:cb + 1])
                    toff = tch * FD
                    if cb == 0:      # k chunk: rope -> k_bf + k_out
                        kro = rope_chunk(k_bf[:, toff:toff + FD], f_sb, toff, FD)
                        nc.sync.dma_start(k_out[:, toff:toff + FD], kro[:, :FD])
                    elif cb == 1:    # v chunk: out + keys-major transpose
                        nc.sync.dma_start(v_out[:, toff:toff + FD], f_sb)
                        for j in range(FD // 128):
                            pt = pss.tile([128, 128], F32, tag="s")
                            nc.tensor.transpose(
                                pt, f_sb[:, j * 128:(j + 1) * 128], ident)
                            nc.vector.tensor_copy(vT_bf[:, tch * 4 + j, :], pt)
                    else:            # q head: rope (q tokens = cols 0:QTOK)
                        hd = cb - 2
                        rope_chunk(q_heads[hd][:, toff:toff + FD], f_sb, toff, FD)

            # ---- attention ----
            for tc_i in range(TC):
                if with_mask:
                    msk = maskp.tile([128, KB, FD], F32, tag="msk")
                    for kb in range(KB):
                        nc.sync.dma_start(
                            msk[:, kb, :],
                            maskT[kb * 128:(kb + 1) * 128,
                                  tc_i * FD:(tc_i + 1) * FD])
                for hd in range(NHL):
                    ps_sum = pssum.tile([128, FD], F32, tag="sum")
                    ps_o = pspv.tile([128, FD], F32, tag="pv")
                    for kb in range(KB):
                        ps_s = pss.tile([128, FD], F32, tag="s")
                        nc.tensor.matmul(ps_s, k_bf[:, kb * 128:(kb + 1) * 128],
                                         q_heads[hd][:, tc_i * FD:(tc_i + 1) * FD])
                        ex = expp.tile([128, FD], BF16, tag="exp")
                        if with_mask:
                            sm = expp.tile([128, FD], F32, tag="smask")
                            nc.vector.tensor_add(sm, ps_s, msk[:, kb, :])
                            nc.scalar.activation(ex, sm, EXP)
                        else:
                            nc.scalar.activation(ex, ps_s, EXP)
                        nc.tensor.matmul(ps_sum, ones_mat, ex,
                                         start=(kb == 0), stop=(kb == KB - 1))
                        nc.tensor.matmul(ps_o, vT_bf[:, kb, :], ex,
                                         start=(kb == 0), stop=(kb == KB - 1))
                    recb = work.tile([128, FD], F32, tag="recb")
                    nc.vector.reciprocal_approx_fast(recb, ps_sum)
                    nc.vector.tensor_mul(
                        att_bf[:, hd, tc_i * FD:(tc_i + 1) * FD], ps_o, recb)

            # ---- o_proj partial: outp = att.T @ o_w ----
            for nb in range(NSB):
                ow = owrp.tile([128, NHL, FD], BF16, tag="ow")
                nc.sync.dma_start(ow, o_wb[nb].rearrange("h p f -> p h f"))
                for tb in range(QTOK // 128):
                    ps = pss.tile([128, FD], F32, tag="s")
                    for hd in range(NHL):
                        nc.tensor.matmul(
                            ps, att_bf[:, hd, tb * 128:(tb + 1) * 128],
                            ow[:, hd, :], start=(hd == 0), stop=(hd == NHL - 1))
                    oc = outcp.tile([128, FD], F32, tag="oc")
                    nc.scalar.activation(oc, ps, COPY)
                    nc.sync.dma_start(
                        outp[tb * 128:(tb + 1) * 128, nb * FD:(nb + 1) * FD], oc)

    nc.compile()
    return nc


_NC_CACHE = {}


def kernel(x, freqs_cis, attention_mask, qkv_w, qkv_b, o_w):
    global LAST_EXEC_NS
    x = np.asarray(x, np.float32)
    freqs_cis = np.asarray(freqs_cis, np.float32)
    attention_mask = np.asarray(attention_mask, np.float32)
    qkv_w = np.asarray(qkv_w, np.float32)
    qkv_b = np.asarray(qkv_b, np.float32)
    o_w = np.asarray(o_w, np.float32)

    with_mask = bool(attention_mask.any())
    if with_mask not in _NC_CACHE:
        _NC_CACHE[with_mask] = build_nc(with_mask)
    nc = _NC_CACHE[with_mask]

    perm = np.r_[0:D:2, 1:D:2]            # evens, odds
    inv = np.argsort(perm)
    scale = 1.0 / math.sqrt(D)

    xf = x.reshape(NTOK, NS)
    in_maps = []
    core_meta = []
    for c in range(8):
        h, t = c // 4, c % 4
        b, sh = t // 2, t % 2
        qlo = sh * 1024
        # per-core token order: q tokens first, then the rest of the batch
        tok_idx = np.r_[qlo:qlo + QTOK, (1024 - qlo):(1024 - qlo) + QTOK]
        gtok = b * S + tok_idx              # global token ids, this batch
        xT_c = xf[gtok].T.astype(ml_dtypes.bfloat16)   # [NS, KTOK]
        # -> [tch, half, kkh, p, f]
        x_all = np.ascontiguousarray(
            xT_c.reshape(4, NSK // 4, 128, TCK, FD).transpose(3, 0, 1, 2, 4))

        # fused weight col-blocks in device order [k, v, q0..q15]
        krows = qkv_w[NH * D + 128 * h:NH * D + 128 * (h + 1)][perm]
        vrows = qkv_w[NG * D + 128 * h:NG * D + 128 * (h + 1)]
        qrows = (qkv_w[2048 * h:2048 * (h + 1)]
                 .reshape(NHL, D, NS)[:, perm] * scale)
        wcols = np.concatenate([krows[None], vrows[None], qrows], 0)  # [18,128,NS]
        # [cb, kk, p(ns), c]
        w_all = np.ascontiguousarray(
            wcols.transpose(2, 0, 1).reshape(NSK, 128, NCB, 128)
            .transpose(2, 0, 1, 3).astype(ml_dtypes.bfloat16))

        bk = qkv_b[NH * D + 128 * h:NH * D + 128 * (h + 1)][perm]
        bv = qkv_b[NG * D + 128 * h:NG * D + 128 * (h + 1)]
        bq = (qkv_b[2048 * h:2048 * (h + 1)].reshape(NHL, D)[:, perm]
              * scale)
        b_all = np.ascontiguousarray(
            np.concatenate([bk[None], bv[None], bq], 0).T.astype(np.float32))

        fc = freqs_cis[b, :, 0, 0]          # [S, 64, 2]
        cos = fc[tok_idx, :, 0].T           # [64, KTOK]
        sin = fc[tok_idx, :, 1].T
        ck = np.concatenate([cos, cos], 0).astype(ml_dtypes.bfloat16)
        sk = np.concatenate([-sin, sin], 0).astype(ml_dtypes.bfloat16)

        # o_w cols for this head-half -> [nb, hd, p(d), f(ns)]
        o_wT = o_w[:, 2048 * h:2048 * (h + 1)].T      # [2048, NS]
        o_wb = np.ascontiguousarray(
            o_wT.reshape(NHL, 128, NSB, FD).transpose(2, 0, 1, 3)
            .astype(ml_dtypes.bfloat16))

        m = dict(x_all=x_all, w_all=w_all, b_all=b_all,
                 ck=np.ascontiguousarray(ck), sk=np.ascontiguousarray(sk),
                 o_wb=o_wb)
        if with_mask:
            mT = attention_mask[b].T        # [key, qpos]
            m["maskT"] = np.ascontiguousarray(
                mT[tok_idx][:, qlo:qlo + QTOK].astype(np.float32))
        in_maps.append(m)
        core_meta.append((h, t, b, sh, gtok))

    res = run_bass_kernel_spmd(nc, in_maps, list(range(8)))
    LAST_EXEC_NS = res.exec_time_ns
    r = res.results

    out = np.empty((NTOK, NS), np.float32)
    for t in range(4):
        b, sh = t // 2, t % 2
        rows = r[t]["outp"] + r[4 + t]["outp"]
        out[b * S + sh * 1024: b * S + sh * 1024 + QTOK] = rows
    out = out.reshape(B, S, NS)

    k_full = np.empty((B, S, NG, 1, D), np.float32)
    v_full = np.empty((B, S, NG, 1, D), np.float32)
    for g in range(2):
        for b in range(2):
            c = g * 4 + 2 * b
            _, _, _, sh, gtok = core_meta[c]
            sidx = gtok - b * S             # seq positions, per-core col order
            k_full[b, sidx, g, 0, :] = r[c]["k_out"][inv].T
            v_full[b, sidx, g, 0, :] = r[c]["v_out"].T
    return out, k_full, v_full


# revision 18
# speedup vs baseline: 1.0080x; 1.0080x over previous
"""ChatGLM2 attention on 8 TRN2 NeuronCores.

Sharding (SPMD, identical graph on all 8 cores; all per-core variation is in
host-prepared input values):
  core c = h*4 + t
    h in {0,1}: head-half = KV group (16 q-heads each)
    t in {0..3}: query-token block: batch b = t//2, seq half t%2 (1024 tokens)
  Each core computes fused-QKV for its columns, RoPE, attention for its
  (16 heads x 1024 q-tokens x 2048 keys), and the o_proj partial for its
  head-half. Host sums the two head-half partials per token block.

Layout: everything "transposed" (feature-major).  fused^T = W^T.T @ x^T gives
q,k d-major directly; scores are computed as S^T = K @ Q^T tiles
[128 keys x 512 toks]; softmax sums via ones-matmul; P@V via
lhsT = V (keys-major, from 16 PE transposes), rhs = exp(S^T).
The head_dim of q/k (and rope tables) is permuted to [evens, odds] so the
RoPE pair-swap is two contiguous 64-partition copies.  The 1/sqrt(128)
q-scale is folded into the q weights/bias host-side.  All DRAM inputs are
pre-blocked host-side so each device load is one big multi-dim DMA.
"""

import math

import numpy as np
import ml_dtypes

import concourse.mybir as mybir
import concourse.tile as tile
from concourse import bacc
from concourse.bass_utils import run_bass_kernel_spmd
from concourse.masks import make_identity

B, S, NS = 2, 2048, 4096
NH, D, NG = 32, 128, 2
NTOK = B * S           # 4096
QTOK = 1024            # q tokens per core
KTOK = 2048            # keys (= batch tokens) per core
NHL = 16               # heads per core
FD = 512               # matmul free dim / psum bank
KB = KTOK // 128       # 16 key blocks
TC = QTOK // FD        # 2 q token chunks
TCK = KTOK // FD       # 4 batch token chunks
NSB = NS // FD         # 8 n_state chunks
NSK = NS // 128        # 32 contraction chunks
NCB = 2 + NHL          # fused col blocks: k, v, q0..q15

F32 = mybir.dt.float32
BF16 = mybir.dt.bfloat16

LAST_EXEC_NS = None


def build_nc(with_mask: bool):
    nc = bacc.Bacc(None, target_bir_lowering=False)

    # x blocked [tch, half, kkh, p, f]
    x_all = nc.declare_dram_parameter("x_all", [TCK, 4, NSK // 4, 128, FD],
                                      BF16, isOutput=False)
    # fused weights blocked [colblk(k,v,q0..15), kk, p(ns), c(col)]
    w_all = nc.declare_dram_parameter("w_all", [NCB, NSK, 128, 128],
                                      BF16, isOutput=False)
    b_all = nc.declare_dram_parameter("b_all", [D, NCB], F32, isOutput=False)
    ck = nc.declare_dram_parameter("ck", [D, KTOK], BF16, isOutput=False)
    sk = nc.declare_dram_parameter("sk", [D, KTOK], BF16, isOutput=False)
    # o_w blocked [nb, hd, p(d), f(ns)]
    o_wb = nc.declare_dram_parameter("o_wb", [NSB, NHL, 128, FD],
                                     BF16, isOutput=False)
    if with_mask:
        maskT = nc.declare_dram_parameter("maskT", [KTOK, QTOK], BF16,
                                          isOutput=False)

    outp = nc.declare_dram_parameter("outp", [QTOK, NS], F32, isOutput=True)
    k_out = nc.declare_dram_parameter("k_out", [D, KTOK], F32, isOutput=True)
    v_out = nc.declare_dram_parameter("v_out", [D, KTOK], F32, isOutput=True)

    COPY = mybir.ActivationFunctionType.Copy
    EXP = mybir.ActivationFunctionType.Exp

    with tile.TileContext(nc) as tc:
        with (
            tc.tile_pool(name="const", bufs=1) as constp,
            tc.tile_pool(name="persist", bufs=1) as persist,
            tc.tile_pool(name="work", bufs=2) as work,
            tc.tile_pool(name="wq", bufs=2) as wqp,
            tc.tile_pool(name="xrhs", bufs=(4 if with_mask else 5)) as xrhsp,
            tc.tile_pool(name="owr", bufs=2) as owrp,
            tc.tile_pool(name="maskp", bufs=1) as maskp,
            tc.tile_pool(name="expp", bufs=4) as expp,
            tc.tile_pool(name="outc", bufs=3) as outcp,
            tc.tile_pool(name="pss", bufs=4, space="PSUM") as pss,
            tc.tile_pool(name="pssum", bufs=2, space="PSUM") as pssum,
            tc.tile_pool(name="pspv", bufs=2, space="PSUM") as pspv,
        ):
            # ---- constants ----
            ident = constp.tile([128, 128], F32)
            make_identity(nc, ident)
            ones_mat = constp.tile([128, 128], BF16)
            nc.gpsimd.memset(ones_mat, 1.0)
            b_sb = constp.tile([D, NCB], F32)
            nc.sync.dma_start(b_sb, b_all[:, :])
            ck_sb = constp.tile([D, KTOK], BF16)
            nc.sync.dma_start(ck_sb, ck[:, :])
            sk_sb = constp.tile([D, KTOK], BF16)
            nc.sync.dma_start(sk_sb, sk[:, :])

            # ---- persistent activations ----
            k_bf = persist.tile([128, KTOK], BF16)       # roped k (perm d-major)
            vT_bf = persist.tile([128, KB, 128], BF16)   # v keys-major chunks
            q_heads = [persist.tile([128, QTOK], BF16, tag=f"qh{i}",
                                    name=f"qh{i}")
                       for i in range(NHL)]
            att_bf = persist.tile([128, NHL, QTOK], BF16)

            def rope_chunk(dst_bf, src_f32, toff, n):
                """dst = (src*cos + swap(src)*sin) -> bf16; cols
                toff:toff+n of the rope tables. src [128, n] f32 perm-d."""
                sw = work.tile([128, FD], F32, tag="ropesw")
                nc.vector.tensor_copy(sw[0:64, :n], src_f32[64:128, :])
                nc.vector.tensor_copy(sw[64:128, :n], src_f32[0:64, :])
                t1 = work.tile([128, FD], F32, tag="ropet1")
                nc.vector.tensor_mul(t1[:, :n], src_f32, ck_sb[:, toff:toff + n])
                nc.vector.tensor_mul(sw[:, :n], sw[:, :n], sk_sb[:, toff:toff + n])
                nc.vector.tensor_add(t1[:, :n], t1[:, :n], sw[:, :n])
                nc.vector.tensor_copy(dst_bf, t1[:, :n])
                return t1

            # ---- fused QKV pass, streamed over batch-token chunks ----
            # tch order: kv-only chunks first so k/v finish early.
            for tch in (0, 1, 2, 3):
                xh = []
                for qt in range(4):
                    xt = xrhsp.tile([128, NSK // 4, FD], BF16, tag="xr")
                    nc.sync.dma_start(
                        xt, x_all[tch, qt].rearrange("k p f -> p k f"))
                    xh.append(xt)
                cbs = (list(range(2, NCB)) + [0, 1]) if tch < TC else [0, 1]
                for cb in cbs:
                    wt = wqp.tile([128, NSK, 128], BF16, tag="w")
                    nc.sync.dma_start(
                        wt, w_all[cb].rearrange("k p c -> p k c"))
                    ps = pss.tile([128, FD], F32, tag="s")
                    for kk in range(NSK):
                        nc.tensor.matmul(ps, wt[:, kk, :],
                                         xh[kk // 8][:, kk % 8, :],
                                         start=(kk == 0), stop=(kk == NSK - 1))
                    f_sb = work.tile([128, FD], F32, tag="fsb")
                    nc.vector.tensor_scalar_add(f_sb, ps, b_sb[:, cb:cb + 1])
                    toff = tch * FD
                    if cb == 0:      # k chunk: rope -> k_bf + k_out
                        kro = rope_chunk(k_bf[:, toff:toff + FD], f_sb, toff, FD)
                        nc.sync.dma_start(k_out[:, toff:toff + FD], kro[:, :FD])
                    elif cb == 1:    # v chunk: out + keys-major transpose
                        nc.sync.dma_start(v_out[:, toff:toff + FD], f_sb)
                        for j in range(FD // 128):
                            pt = pss.tile([128, 128], F32, tag="s")
                            nc.tensor.transpose(
                                pt, f_sb[:, j * 128:(j + 1) * 128], ident)
                            nc.vector.tensor_copy(vT_bf[:, tch * 4 + j, :], pt)
                    else:            # q head: rope (q tokens = cols 0:QTOK)
                        hd = cb - 2
                        rope_chunk(q_heads[hd][:, toff:toff + FD], f_sb, toff, FD)

            # ---- attention ----
            for tc_i in range(TC):
                if with_mask:
                    msk = maskp.tile([128, KB, FD], BF16, tag="msk")
                    for kb in range(KB):
                        nc.sync.dma_start(
                            msk[:, kb, :],
                            maskT[kb * 128:(kb + 1) * 128,
                                  tc_i * FD:(tc_i + 1) * FD])
                for hd in range(NHL):
                    ps_sum = pssum.tile([128, FD], F32, tag="sum")
                    ps_o = pspv.tile([128, FD], F32, tag="pv")
                    for kb in range(KB):
                        ps_s = pss.tile([128, FD], F32, tag="s")
                        nc.tensor.matmul(ps_s, k_bf[:, kb * 128:(kb + 1) * 128],
                                         q_heads[hd][:, tc_i * FD:(tc_i + 1) * FD])
                        ex = expp.tile([128, FD], BF16, tag="exp")
                        if with_mask:
                            sm = maskp.tile([128, FD], BF16, tag="smask")
                            nc.vector.tensor_add(sm, ps_s, msk[:, kb, :])
                            nc.scalar.activation(ex, sm, EXP)
                        else:
                            nc.scalar.activation(ex, ps_s, EXP)
                        nc.tensor.matmul(ps_sum, ones_mat, ex,
                                         start=(kb == 0), stop=(kb == KB - 1))
                        nc.tensor.matmul(ps_o, vT_bf[:, kb, :], ex,
                                         start=(kb == 0), stop=(kb == KB - 1))
                    recb = work.tile([128, FD], F32, tag="recb")
                    nc.vector.reciprocal_approx_fast(recb, ps_sum)
                    nc.vector.tensor_mul(
                        att_bf[:, hd, tc_i * FD:(tc_i + 1) * FD], ps_o, recb)

            # ---- o_proj partial: outp = att.T @ o_w ----
            for nb in range(NSB):
                ow = owrp.tile([128, NHL, FD], BF16, tag="ow")
                nc.sync.dma_start(ow, o_wb[nb].rearrange("h p f -> p h f"))
                for tb in range(QTOK // 128):
                    ps = pss.tile([128, FD], F32, tag="s")
                    for hd in range(NHL):
                        nc.tensor.matmul(
                            ps, att_bf[:, hd, tb * 128:(tb + 1) * 128],
                            ow[:, hd, :], start=(hd == 0), stop=(hd == NHL - 1))
                    oc = outcp.tile([128, FD], F32, tag="oc")
                    nc.scalar.activation(oc, ps, COPY)
                    nc.sync.dma_start(
                        outp[tb * 128:(tb + 1) * 128, nb * FD:(nb + 1) * FD], oc)

    nc.compile()
    return nc


_NC_CACHE = {}


def kernel(x, freqs_cis, attention_mask, qkv_w, qkv_b, o_w):
    global LAST_EXEC_NS
    x = np.asarray(x, np.float32)
    freqs_cis = np.asarray(freqs_cis, np.float32)
    attention_mask = np.asarray(attention_mask, np.float32)
    qkv_w = np.asarray(qkv_w, np.float32)
    qkv_b = np.asarray(qkv_b, np.float32)
    o_w = np.asarray(o_w, np.float32)

    with_mask = bool(attention_mask.any())
    if with_mask not in _NC_CACHE:
        _NC_CACHE[with_mask] = build_nc(with_mask)
    nc = _NC_CACHE[with_mask]

    perm = np.r_[0:D:2, 1:D:2]            # evens, odds
    inv = np.argsort(perm)
    scale = 1.0 / math.sqrt(D)

    xf = x.reshape(NTOK, NS)
    in_maps = []
    core_meta = []
    for c in range(8):
        h, t = c // 4, c % 4
        b, sh = t // 2, t % 2
        qlo = sh * 1024
        # per-core token order: q tokens first, then the rest of the batch
        tok_idx = np.r_[qlo:qlo + QTOK, (1024 - qlo):(1024 - qlo) + QTOK]
        gtok = b * S + tok_idx              # global token ids, this batch
        xT_c = xf[gtok].T.astype(ml_dtypes.bfloat16)   # [NS, KTOK]
        # -> [tch, half, kkh, p, f]
        x_all = np.ascontiguousarray(
            xT_c.reshape(4, NSK // 4, 128, TCK, FD).transpose(3, 0, 1, 2, 4))

        # fused weight col-blocks in device order [k, v, q0..q15]
        krows = qkv_w[NH * D + 128 * h:NH * D + 128 * (h + 1)][perm]
        vrows = qkv_w[NG * D + 128 * h:NG * D + 128 * (h + 1)]
        qrows = (qkv_w[2048 * h:2048 * (h + 1)]
                 .reshape(NHL, D, NS)[:, perm] * scale)
        wcols = np.concatenate([krows[None], vrows[None], qrows], 0)  # [18,128,NS]
        # [cb, kk, p(ns), c]
        w_all = np.ascontiguousarray(
            wcols.transpose(2, 0, 1).reshape(NSK, 128, NCB, 128)
            .transpose(2, 0, 1, 3).astype(ml_dtypes.bfloat16))

        bk = qkv_b[NH * D + 128 * h:NH * D + 128 * (h + 1)][perm]
        bv = qkv_b[NG * D + 128 * h:NG * D + 128 * (h + 1)]
        bq = (qkv_b[2048 * h:2048 * (h + 1)].reshape(NHL, D)[:, perm]
              * scale)
        b_all = np.ascontiguousarray(
            np.concatenate([bk[None], bv[None], bq], 0).T.astype(np.float32))

        fc = freqs_cis[b, :, 0, 0]          # [S, 64, 2]
        cos = fc[tok_idx, :, 0].T           # [64, KTOK]
        sin = fc[tok_idx, :, 1].T
        ck = np.concatenate([cos, cos], 0).astype(ml_dtypes.bfloat16)
        sk = np.concatenate([-sin, sin], 0).astype(ml_dtypes.bfloat16)

        # o_w cols for this head-half -> [nb, hd, p(d), f(ns)]
        o_wT = o_w[:, 2048 * h:2048 * (h + 1)].T      # [2048, NS]
        o_wb = np.ascontiguousarray(
            o_wT.reshape(NHL, 128, NSB, FD).transpose(2, 0, 1, 3)
            .astype(ml_dtypes.bfloat16))

        m = dict(x_all=x_all, w_all=w_all, b_all=b_all,
                 ck=np.ascontiguousarray(ck), sk=np.ascontiguousarray(sk),
                 o_wb=o_wb)
        if with_mask:
            mT = attention_mask[b].T        # [key, qpos]
            m["maskT"] = np.ascontiguousarray(
                mT[tok_idx][:, qlo:qlo + QTOK].astype(ml_dtypes.bfloat16))
        in_maps.append(m)
        core_meta.append((h, t, b, sh, gtok))

    res = run_bass_kernel_spmd(nc, in_maps, list(range(8)))
    LAST_EXEC_NS = res.exec_time_ns
    r = res.results

    out = np.empty((NTOK, NS), np.float32)
    for t in range(4):
        b, sh = t // 2, t % 2
        rows = r[t]["outp"] + r[4 + t]["outp"]
        out[b * S + sh * 1024: b * S + sh * 1024 + QTOK] = rows
    out = out.reshape(B, S, NS)

    k_full = np.empty((B, S, NG, 1, D), np.float32)
    v_full = np.empty((B, S, NG, 1, D), np.float32)
    for g in range(2):
        for b in range(2):
            c = g * 4 + 2 * b
            _, _, _, sh, gtok = core_meta[c]
            sidx = gtok - b * S             # seq positions, per-core col order
            k_full[b, sidx, g, 0, :] = r[c]["k_out"][inv].T
            v_full[b, sidx, g, 0, :] = r[c]["v_out"].T
    return out, k_full, v_full


# revision 19
# speedup vs baseline: 1.0311x; 1.0229x over previous
"""ChatGLM2 attention on 8 TRN2 NeuronCores.

Sharding (SPMD, identical graph on all 8 cores; all per-core variation is in
host-prepared input values):
  core c = h*4 + t
    h in {0,1}: head-half = KV group (16 q-heads each)
    t in {0..3}: query-token block: batch b = t//2, seq half t%2 (1024 tokens)
  Each core computes fused-QKV for its columns, RoPE, attention for its
  (16 heads x 1024 q-tokens x 2048 keys), and the o_proj partial for its
  head-half. Host sums the two head-half partials per token block.

Layout: everything "transposed" (feature-major).  fused^T = W^T.T @ x^T gives
q,k d-major directly; scores are computed as S^T = K @ Q^T tiles
[128 keys x 512 toks]; softmax sums via ones-matmul; P@V via
lhsT = V (keys-major, from 16 PE transposes), rhs = exp(S^T).
The head_dim of q/k (and rope tables) is permuted to [evens, odds] so the
RoPE pair-swap is two contiguous 64-partition copies.  The 1/sqrt(128)
q-scale is folded into the q weights/bias host-side.  All DRAM inputs are
pre-blocked host-side so each device load is one big multi-dim DMA.
"""

import math

import numpy as np
import ml_dtypes

import concourse.mybir as mybir
import concourse.tile as tile
from concourse import bacc
from concourse.bass_utils import run_bass_kernel_spmd
from concourse.masks import make_identity

B, S, NS = 2, 2048, 4096
NH, D, NG = 32, 128, 2
NTOK = B * S           # 4096
QTOK = 1024            # q tokens per core
KTOK = 2048            # keys (= batch tokens) per core
NHL = 16               # heads per core
FD = 512               # matmul free dim / psum bank
KB = KTOK // 128       # 16 key blocks
TC = QTOK // FD        # 2 q token chunks
TCK = KTOK // FD       # 4 batch token chunks
NSB = NS // FD         # 8 n_state chunks
NSK = NS // 128        # 32 contraction chunks
NCB = 2 + NHL          # fused col blocks: k, v, q0..q15

F32 = mybir.dt.float32
BF16 = mybir.dt.bfloat16

LAST_EXEC_NS = None


def build_nc(with_mask: bool):
    nc = bacc.Bacc(None, target_bir_lowering=False)

    # x blocked [tch, half, kkh, p, f]
    x_all = nc.declare_dram_parameter("x_all", [TCK, 4, NSK // 4, 128, FD],
                                      BF16, isOutput=False)
    # fused weights blocked [colblk(k,v,q0..15), kk, p(ns), c(col)]
    w_all = nc.declare_dram_parameter("w_all", [NCB, NSK, 128, 128],
                                      BF16, isOutput=False)
    b_all = nc.declare_dram_parameter("b_all", [D, NCB], F32, isOutput=False)
    ck = nc.declare_dram_parameter("ck", [D, KTOK], BF16, isOutput=False)
    sk = nc.declare_dram_parameter("sk", [D, KTOK], BF16, isOutput=False)
    # o_w blocked [nb, hd, p(d), f(ns)]
    o_wb = nc.declare_dram_parameter("o_wb", [NSB, NHL, 128, FD],
                                     BF16, isOutput=False)
    if with_mask:
        maskT = nc.declare_dram_parameter("maskT", [KTOK, QTOK], BF16,
                                          isOutput=False)

    outp = nc.declare_dram_parameter("outp", [QTOK, NS], F32, isOutput=True)
    k_out = nc.declare_dram_parameter("k_out", [D, KTOK], F32, isOutput=True)
    v_out = nc.declare_dram_parameter("v_out", [D, KTOK], F32, isOutput=True)

    COPY = mybir.ActivationFunctionType.Copy
    EXP = mybir.ActivationFunctionType.Exp

    with tile.TileContext(nc) as tc:
        with (
            tc.tile_pool(name="const", bufs=1) as constp,
            tc.tile_pool(name="persist", bufs=1) as persist,
            tc.tile_pool(name="work", bufs=2) as work,
            tc.tile_pool(name="wq", bufs=3) as wqp,
            tc.tile_pool(name="xrhs", bufs=(4 if with_mask else 5)) as xrhsp,
            tc.tile_pool(name="owr", bufs=2) as owrp,
            tc.tile_pool(name="maskp", bufs=1) as maskp,
            tc.tile_pool(name="expp", bufs=4) as expp,
            tc.tile_pool(name="outc", bufs=3) as outcp,
            tc.tile_pool(name="pss", bufs=4, space="PSUM") as pss,
            tc.tile_pool(name="pssum", bufs=2, space="PSUM") as pssum,
            tc.tile_pool(name="pspv", bufs=2, space="PSUM") as pspv,
        ):
            # ---- constants ----
            ident = constp.tile([128, 128], F32)
            make_identity(nc, ident)
            ones_mat = constp.tile([128, 128], BF16)
            nc.gpsimd.memset(ones_mat, 1.0)
            b_sb = constp.tile([D, NCB], F32)
            nc.sync.dma_start(b_sb, b_all[:, :])
            ck_sb = constp.tile([D, KTOK], BF16)
            nc.sync.dma_start(ck_sb, ck[:, :])
            sk_sb = constp.tile([D, KTOK], BF16)
            nc.sync.dma_start(sk_sb, sk[:, :])

            # ---- persistent activations ----
            k_bf = persist.tile([128, KTOK], BF16)       # roped k (perm d-major)
            vT_bf = persist.tile([128, KB, 128], BF16)   # v keys-major chunks
            q_heads = [persist.tile([128, QTOK], BF16, tag=f"qh{i}",
                                    name=f"qh{i}")
                       for i in range(NHL)]
            att_bf = persist.tile([128, NHL, QTOK], BF16)

            def rope_chunk(dst_bf, src_f32, toff, n):
                """dst = (src*cos + swap(src)*sin) -> bf16; cols
                toff:toff+n of the rope tables. src [128, n] f32 perm-d."""
                sw = work.tile([128, FD], F32, tag="ropesw")
                nc.vector.tensor_copy(sw[0:64, :n], src_f32[64:128, :])
                nc.vector.tensor_copy(sw[64:128, :n], src_f32[0:64, :])
                t1 = work.tile([128, FD], F32, tag="ropet1")
                nc.vector.tensor_mul(t1[:, :n], src_f32, ck_sb[:, toff:toff + n])
                nc.vector.tensor_mul(sw[:, :n], sw[:, :n], sk_sb[:, toff:toff + n])
                nc.vector.tensor_add(t1[:, :n], t1[:, :n], sw[:, :n])
                nc.vector.tensor_copy(dst_bf, t1[:, :n])
                return t1

            # ---- fused QKV pass, streamed over batch-token chunks ----
            # tch order: kv-only chunks first so k/v finish early.
            for tch in (0, 1, 2, 3):
                xh = []
                for qt in range(4):
                    xt = xrhsp.tile([128, NSK // 4, FD], BF16, tag="xr")
                    nc.sync.dma_start(
                        xt, x_all[tch, qt].rearrange("k p f -> p k f"))
                    xh.append(xt)
                cbs = (list(range(2, NCB)) + [0, 1]) if tch < TC else [0, 1]
                for cb in cbs:
                    wt = wqp.tile([128, NSK, 128], BF16, tag="w")
                    nc.sync.dma_start(
                        wt, w_all[cb].rearrange("k p c -> p k c"))
                    ps = pss.tile([128, FD], F32, tag="s")
                    for kk in range(NSK):
                        nc.tensor.matmul(ps, wt[:, kk, :],
                                         xh[kk // 8][:, kk % 8, :],
                                         start=(kk == 0), stop=(kk == NSK - 1))
                    f_sb = work.tile([128, FD], F32, tag="fsb")
                    nc.vector.tensor_scalar_add(f_sb, ps, b_sb[:, cb:cb + 1])
                    toff = tch * FD
                    if cb == 0:      # k chunk: rope -> k_bf + k_out
                        kro = rope_chunk(k_bf[:, toff:toff + FD], f_sb, toff, FD)
                        nc.sync.dma_start(k_out[:, toff:toff + FD], kro[:, :FD])
                    elif cb == 1:    # v chunk: out + keys-major transpose
                        nc.sync.dma_start(v_out[:, toff:toff + FD], f_sb)
                        for j in range(FD // 128):
                            pt = pss.tile([128, 128], F32, tag="s")
                            nc.tensor.transpose(
                                pt, f_sb[:, j * 128:(j + 1) * 128], ident)
                            nc.vector.tensor_copy(vT_bf[:, tch * 4 + j, :], pt)
                    else:            # q head: rope (q tokens = cols 0:QTOK)
                        hd = cb - 2
                        rope_chunk(q_heads[hd][:, toff:toff + FD], f_sb, toff, FD)

            # ---- attention ----
            for tc_i in range(TC):
                if with_mask:
                    msk = maskp.tile([128, KB, FD], BF16, tag="msk")
                    for kb in range(KB):
                        nc.sync.dma_start(
                            msk[:, kb, :],
                            maskT[kb * 128:(kb + 1) * 128,
                                  tc_i * FD:(tc_i + 1) * FD])
                for hd in range(NHL):
                    ps_sum = pssum.tile([128, FD], F32, tag="sum")
                    ps_o = pspv.tile([128, FD], F32, tag="pv")
                    for kb in range(KB):
                        ps_s = pss.tile([128, FD], F32, tag="s")
                        nc.tensor.matmul(ps_s, k_bf[:, kb * 128:(kb + 1) * 128],
                                         q_heads[hd][:, tc_i * FD:(tc_i + 1) * FD])
                        ex = expp.tile([128, FD], BF16, tag="exp")
                        if with_mask:
                            sm = maskp.tile([128, FD], BF16, tag="smask")
                            nc.vector.tensor_add(sm, ps_s, msk[:, kb, :])
                            nc.scalar.activation(ex, sm, EXP)
                        else:
                            nc.scalar.activation(ex, ps_s, EXP)
                        nc.tensor.matmul(ps_sum, ones_mat, ex,
                                         start=(kb == 0), stop=(kb == KB - 1))
                        nc.tensor.matmul(ps_o, vT_bf[:, kb, :], ex,
                                         start=(kb == 0), stop=(kb == KB - 1))
                    recb = work.tile([128, FD], F32, tag="recb")
                    nc.vector.reciprocal_approx_fast(recb, ps_sum)
                    nc.vector.tensor_mul(
                        att_bf[:, hd, tc_i * FD:(tc_i + 1) * FD], ps_o, recb)

            # ---- o_proj partial: outp = att.T @ o_w ----
            for nb in range(NSB):
                ow = owrp.tile([128, NHL, FD], BF16, tag="ow")
                nc.sync.dma_start(ow, o_wb[nb].rearrange("h p f -> p h f"))
                for tb in range(QTOK // 128):
                    ps = pss.tile([128, FD], F32, tag="s")
                    for hd in range(NHL):
                        nc.tensor.matmul(
                            ps, att_bf[:, hd, tb * 128:(tb + 1) * 128],
                            ow[:, hd, :], start=(hd == 0), stop=(hd == NHL - 1))
                    oc = outcp.tile([128, FD], F32, tag="oc")
                    nc.scalar.activation(oc, ps, COPY)
                    nc.sync.dma_start(
                        outp[tb * 128:(tb + 1) * 128, nb * FD:(nb + 1) * FD], oc)

    nc.compile()
    return nc


_NC_CACHE = {}


def kernel(x, freqs_cis, attention_mask, qkv_w, qkv_b, o_w):
    global LAST_EXEC_NS
    x = np.asarray(x, np.float32)
    freqs_cis = np.asarray(freqs_cis, np.float32)
    attention_mask = np.asarray(attention_mask, np.float32)
    qkv_w = np.asarray(qkv_w, np.float32)
    qkv_b = np.asarray(qkv_b, np.float32)
    o_w = np.asarray(o_w, np.float32)

    with_mask = bool(attention_mask.any())
    if with_mask not in _NC_CACHE:
        _NC_CACHE[with_mask] = build_nc(with_mask)
    nc = _NC_CACHE[with_mask]

    perm = np.r_[0:D:2, 1:D:2]            # evens, odds
    inv = np.argsort(perm)
    scale = 1.0 / math.sqrt(D)

    xf = x.reshape(NTOK, NS)
    in_maps = []
    core_meta = []
    for c in range(8):
        h, t = c // 4, c % 4
        b, sh = t // 2, t % 2
        qlo = sh * 1024
        # per-core token order: q tokens first, then the rest of the batch
        tok_idx = np.r_[qlo:qlo + QTOK, (1024 - qlo):(1024 - qlo) + QTOK]
        gtok = b * S + tok_idx              # global token ids, this batch
        xT_c = xf[gtok].T.astype(ml_dtypes.bfloat16)   # [NS, KTOK]
        # -> [tch, half, kkh, p, f]
        x_all = np.ascontiguousarray(
            xT_c.reshape(4, NSK // 4, 128, TCK, FD).transpose(3, 0, 1, 2, 4))

        # fused weight col-blocks in device order [k, v, q0..q15]
        krows = qkv_w[NH * D + 128 * h:NH * D + 128 * (h + 1)][perm]
        vrows = qkv_w[NG * D + 128 * h:NG * D + 128 * (h + 1)]
        qrows = (qkv_w[2048 * h:2048 * (h + 1)]
                 .reshape(NHL, D, NS)[:, perm] * scale)
        wcols = np.concatenate([krows[None], vrows[None], qrows], 0)  # [18,128,NS]
        # [cb, kk, p(ns), c]
        w_all = np.ascontiguousarray(
            wcols.transpose(2, 0, 1).reshape(NSK, 128, NCB, 128)
            .transpose(2, 0, 1, 3).astype(ml_dtypes.bfloat16))

        bk = qkv_b[NH * D + 128 * h:NH * D + 128 * (h + 1)][perm]
        bv = qkv_b[NG * D + 128 * h:NG * D + 128 * (h + 1)]
        bq = (qkv_b[2048 * h:2048 * (h + 1)].reshape(NHL, D)[:, perm]
              * scale)
        b_all = np.ascontiguousarray(
            np.concatenate([bk[None], bv[None], bq], 0).T.astype(np.float32))

        fc = freqs_cis[b, :, 0, 0]          # [S, 64, 2]
        cos = fc[tok_idx, :, 0].T           # [64, KTOK]
        sin = fc[tok_idx, :, 1].T
        ck = np.concatenate([cos, cos], 0).astype(ml_dtypes.bfloat16)
        sk = np.concatenate([-sin, sin], 0).astype(ml_dtypes.bfloat16)

        # o_w cols for this head-half -> [nb, hd, p(d), f(ns)]
        o_wT = o_w[:, 2048 * h:2048 * (h + 1)].T      # [2048, NS]
        o_wb = np.ascontiguousarray(
            o_wT.reshape(NHL, 128, NSB, FD).transpose(2, 0, 1, 3)
            .astype(ml_dtypes.bfloat16))

        m = dict(x_all=x_all, w_all=w_all, b_all=b_all,
                 ck=np.ascontiguousarray(ck), sk=np.ascontiguousarray(sk),
                 o_wb=o_wb)
        if with_mask:
            mT = attention_mask[b].T        # [key, qpos]
            m["maskT"] = np.ascontiguousarray(
                mT[tok_idx][:, qlo:qlo + QTOK].astype(ml_dtypes.bfloat16))
        in_maps.append(m)
        core_meta.append((h, t, b, sh, gtok))

    res = run_bass_kernel_spmd(nc, in_maps, list(range(8)))
    LAST_EXEC_NS = res.exec_time_ns
    r = res.results

    out = np.empty((NTOK, NS), np.float32)
    for t in range(4):
        b, sh = t // 2, t % 2
        rows = r[t]["outp"] + r[4 + t]["outp"]
        out[b * S + sh * 1024: b * S + sh * 1024 + QTOK] = rows
    out = out.reshape(B, S, NS)

    k_full = np.empty((B, S, NG, 1, D), np.float32)
    v_full = np.empty((B, S, NG, 1, D), np.float32)
    for g in range(2):
        for b in range(2):
            c = g * 4 + 2 * b
            _, _, _, sh, gtok = core_meta[c]
            sidx = gtok - b * S             # seq positions, per-core col order
            k_full[b, sidx, g, 0, :] = r[c]["k_out"][inv].T
            v_full[b, sidx, g, 0, :] = r[c]["v_out"].T
    return out, k_full, v_full


# revision 21
# speedup vs baseline: 1.0315x; 1.0004x over previous
"""ChatGLM2 attention on 8 TRN2 NeuronCores.

Sharding (SPMD, identical graph on all 8 cores; all per-core variation is in
host-prepared input values):
  core c = h*4 + t
    h in {0,1}: head-half = KV group (16 q-heads each)
    t in {0..3}: query-token block: batch b = t//2, seq half t%2 (1024 tokens)
  Each core computes fused-QKV for its columns, RoPE, attention for its
  (16 heads x 1024 q-tokens x 2048 keys), and the o_proj partial for its
  head-half. Host sums the two head-half partials per token block.

Layout: everything "transposed" (feature-major).  fused^T = W^T.T @ x^T gives
q,k d-major directly; scores are computed as S^T = K @ Q^T tiles
[128 keys x 512 toks]; softmax sums via ones-matmul; P@V via
lhsT = V (keys-major, from 16 PE transposes), rhs = exp(S^T).
The head_dim of q/k (and rope tables) is permuted to [evens, odds] so the
RoPE pair-swap is two contiguous 64-partition copies.  The 1/sqrt(128)
q-scale is folded into the q weights/bias host-side.  All DRAM inputs are
pre-blocked host-side so each device load is one big multi-dim DMA.
"""

import math

import numpy as np
import ml_dtypes

import concourse.mybir as mybir
import concourse.tile as tile
from concourse import bacc
from concourse.bass_utils import run_bass_kernel_spmd
from concourse.masks import make_identity

B, S, NS = 2, 2048, 4096
NH, D, NG = 32, 128, 2
NTOK = B * S           # 4096
QTOK = 1024            # q tokens per core
KTOK = 2048            # keys (= batch tokens) per core
NHL = 16               # heads per core
FD = 512               # matmul free dim / psum bank
KB = KTOK // 128       # 16 key blocks
TC = QTOK // FD        # 2 q token chunks
TCK = KTOK // FD       # 4 batch token chunks
NSB = NS // FD         # 8 n_state chunks
NSK = NS // 128        # 32 contraction chunks
NCB = 2 + NHL          # fused col blocks: k, v, q0..q15

F32 = mybir.dt.float32
BF16 = mybir.dt.bfloat16

LAST_EXEC_NS = None


def build_nc(with_mask: bool):
    nc = bacc.Bacc(None, target_bir_lowering=False)

    # x blocked [tch, half, kkh, p, f]
    x_all = nc.declare_dram_parameter("x_all", [TCK, 4, NSK // 4, 128, FD],
                                      BF16, isOutput=False)
    # fused weights blocked [colblk(k,v,q0..15), kk, p(ns), c(col)]
    w_all = nc.declare_dram_parameter("w_all", [NCB, NSK, 128, 128],
                                      BF16, isOutput=False)
    b_all = nc.declare_dram_parameter("b_all", [D, NCB], F32, isOutput=False)
    ck = nc.declare_dram_parameter("ck", [D, KTOK], BF16, isOutput=False)
    sk = nc.declare_dram_parameter("sk", [D, KTOK], BF16, isOutput=False)
    # o_w blocked [nb, hd, p(d), f(ns)]
    o_wb = nc.declare_dram_parameter("o_wb", [NSB, NHL, 128, FD],
                                     BF16, isOutput=False)
    if with_mask:
        maskT = nc.declare_dram_parameter("maskT", [KTOK, QTOK], BF16,
                                          isOutput=False)

    outp = nc.declare_dram_parameter("outp", [QTOK, NS], F32, isOutput=True)
    k_out = nc.declare_dram_parameter("k_out", [D, KTOK], F32, isOutput=True)
    v_out = nc.declare_dram_parameter("v_out", [D, KTOK], F32, isOutput=True)

    COPY = mybir.ActivationFunctionType.Copy
    EXP = mybir.ActivationFunctionType.Exp

    with tile.TileContext(nc) as tc:
        with (
            tc.tile_pool(name="const", bufs=1) as constp,
            tc.tile_pool(name="persist", bufs=1) as persist,
            tc.tile_pool(name="work", bufs=2) as work,
            tc.tile_pool(name="wq", bufs=3) as wqp,
            tc.tile_pool(name="xrhs", bufs=(4 if with_mask else 5)) as xrhsp,
            tc.tile_pool(name="owr", bufs=2) as owrp,
            tc.tile_pool(name="maskp", bufs=1) as maskp,
            tc.tile_pool(name="expp", bufs=6) as expp,
            tc.tile_pool(name="outc", bufs=3) as outcp,
            tc.tile_pool(name="pss", bufs=4, space="PSUM") as pss,
            tc.tile_pool(name="pssum", bufs=2, space="PSUM") as pssum,
            tc.tile_pool(name="pspv", bufs=2, space="PSUM") as pspv,
        ):
            # ---- constants ----
            ident = constp.tile([128, 128], F32)
            make_identity(nc, ident)
            ones_mat = constp.tile([128, 128], BF16)
            nc.gpsimd.memset(ones_mat, 1.0)
            b_sb = constp.tile([D, NCB], F32)
            nc.sync.dma_start(b_sb, b_all[:, :])
            ck_sb = constp.tile([D, KTOK], BF16)
            nc.sync.dma_start(ck_sb, ck[:, :])
            sk_sb = constp.tile([D, KTOK], BF16)
            nc.sync.dma_start(sk_sb, sk[:, :])

            # ---- persistent activations ----
            k_bf = persist.tile([128, KTOK], BF16)       # roped k (perm d-major)
            vT_bf = persist.tile([128, KB, 128], BF16)   # v keys-major chunks
            q_heads = [persist.tile([128, QTOK], BF16, tag=f"qh{i}",
                                    name=f"qh{i}")
                       for i in range(NHL)]
            att_bf = persist.tile([128, NHL, QTOK], BF16)

            def rope_chunk(dst_bf, src_f32, toff, n):
                """dst = (src*cos + swap(src)*sin) -> bf16; cols
                toff:toff+n of the rope tables. src [128, n] f32 perm-d."""
                sw = work.tile([128, FD], F32, tag="ropesw")
                nc.vector.tensor_copy(sw[0:64, :n], src_f32[64:128, :])
                nc.vector.tensor_copy(sw[64:128, :n], src_f32[0:64, :])
                t1 = work.tile([128, FD], F32, tag="ropet1")
                nc.vector.tensor_mul(t1[:, :n], src_f32, ck_sb[:, toff:toff + n])
                nc.vector.tensor_mul(sw[:, :n], sw[:, :n], sk_sb[:, toff:toff + n])
                nc.vector.tensor_add(t1[:, :n], t1[:, :n], sw[:, :n])
                nc.vector.tensor_copy(dst_bf, t1[:, :n])
                return t1

            # ---- fused QKV pass, streamed over batch-token chunks ----
            # tch order: kv-only chunks first so k/v finish early.
            for tch in (0, 1, 2, 3):
                xh = []
                for qt in range(4):
                    xt = xrhsp.tile([128, NSK // 4, FD], BF16, tag="xr")
                    nc.sync.dma_start(
                        xt, x_all[tch, qt].rearrange("k p f -> p k f"))
                    xh.append(xt)
                cbs = (list(range(2, NCB)) + [0, 1]) if tch < TC else [0, 1]
                for cb in cbs:
                    wt = wqp.tile([128, NSK, 128], BF16, tag="w")
                    nc.sync.dma_start(
                        wt, w_all[cb].rearrange("k p c -> p k c"))
                    ps = pss.tile([128, FD], F32, tag="s")
                    for kk in range(NSK):
                        nc.tensor.matmul(ps, wt[:, kk, :],
                                         xh[kk // 8][:, kk % 8, :],
                                         start=(kk == 0), stop=(kk == NSK - 1))
                    f_sb = work.tile([128, FD], F32, tag="fsb")
                    nc.vector.tensor_scalar_add(f_sb, ps, b_sb[:, cb:cb + 1])
                    toff = tch * FD
                    if cb == 0:      # k chunk: rope -> k_bf + k_out
                        kro = rope_chunk(k_bf[:, toff:toff + FD], f_sb, toff, FD)
                        nc.sync.dma_start(k_out[:, toff:toff + FD], kro[:, :FD])
                    elif cb == 1:    # v chunk: out + keys-major transpose
                        nc.sync.dma_start(v_out[:, toff:toff + FD], f_sb)
                        for j in range(FD // 128):
                            pt = pss.tile([128, 128], F32, tag="s")
                            nc.tensor.transpose(
                                pt, f_sb[:, j * 128:(j + 1) * 128], ident)
                            nc.vector.tensor_copy(vT_bf[:, tch * 4 + j, :], pt)
                    else:            # q head: rope (q tokens = cols 0:QTOK)
                        hd = cb - 2
                        rope_chunk(q_heads[hd][:, toff:toff + FD], f_sb, toff, FD)

            # ---- attention ----
            for tc_i in range(TC):
                if with_mask:
                    msk = maskp.tile([128, KB, FD], BF16, tag="msk")
                    for kb in range(KB):
                        nc.sync.dma_start(
                            msk[:, kb, :],
                            maskT[kb * 128:(kb + 1) * 128,
                                  tc_i * FD:(tc_i + 1) * FD])
                for hd in range(NHL):
                    ps_sum = pssum.tile([128, FD], F32, tag="sum")
                    ps_o = pspv.tile([128, FD], F32, tag="pv")
                    for kb in range(KB):
                        ps_s = pss.tile([128, FD], F32, tag="s")
                        nc.tensor.matmul(ps_s, k_bf[:, kb * 128:(kb + 1) * 128],
                                         q_heads[hd][:, tc_i * FD:(tc_i + 1) * FD])
                        ex = expp.tile([128, FD], BF16, tag="exp")
                        if with_mask:
                            sm = maskp.tile([128, FD], BF16, tag="smask")
                            nc.vector.tensor_add(sm, ps_s, msk[:, kb, :])
                            nc.scalar.activation(ex, sm, EXP)
                        else:
                            nc.scalar.activation(ex, ps_s, EXP)
                        nc.tensor.matmul(ps_sum, ones_mat, ex,
                                         start=(kb == 0), stop=(kb == KB - 1))
                        nc.tensor.matmul(ps_o, vT_bf[:, kb, :], ex,
                                         start=(kb == 0), stop=(kb == KB - 1))
                    recb = work.tile([128, FD], F32, tag="recb")
                    nc.vector.reciprocal_approx_fast(recb, ps_sum)
                    nc.vector.tensor_mul(
                        att_bf[:, hd, tc_i * FD:(tc_i + 1) * FD], ps_o, recb)

            # ---- o_proj partial: outp = att.T @ o_w ----
            for nb in range(NSB):
                ow = owrp.tile([128, NHL, FD], BF16, tag="ow")
                nc.sync.dma_start(ow, o_wb[nb].rearrange("h p f -> p h f"))
                for tb in range(QTOK // 128):
                    ps = pss.tile([128, FD], F32, tag="s")
                    for hd in range(NHL):
                        nc.tensor.matmul(
                            ps, att_bf[:, hd, tb * 128:(tb + 1) * 128],
                            ow[:, hd, :], start=(hd == 0), stop=(hd == NHL - 1))
                    oc = outcp.tile([128, FD], F32, tag="oc")
                    nc.scalar.activation(oc, ps, COPY)
                    nc.sync.dma_start(
                        outp[tb * 128:(tb + 1) * 128, nb * FD:(nb + 1) * FD], oc)

    nc.compile()
    return nc


_NC_CACHE = {}


def kernel(x, freqs_cis, attention_mask, qkv_w, qkv_b, o_w):
    global LAST_EXEC_NS
    x = np.asarray(x, np.float32)
    freqs_cis = np.asarray(freqs_cis, np.float32)
    attention_mask = np.asarray(attention_mask, np.float32)
    qkv_w = np.asarray(qkv_w, np.float32)
    qkv_b = np.asarray(qkv_b, np.float32)
    o_w = np.asarray(o_w, np.float32)

    with_mask = bool(attention_mask.any())
    if with_mask not in _NC_CACHE:
        _NC_CACHE[with_mask] = build_nc(with_mask)
    nc = _NC_CACHE[with_mask]

    perm = np.r_[0:D:2, 1:D:2]            # evens, odds
    inv = np.argsort(perm)
    scale = 1.0 / math.sqrt(D)

    xf = x.reshape(NTOK, NS)
    in_maps = []
    core_meta = []
    for c in range(8):
        h, t = c // 4, c % 4
        b, sh = t // 2, t % 2
        qlo = sh * 1024
        # per-core token order: q tokens first, then the rest of the batch
        tok_idx = np.r_[qlo:qlo + QTOK, (1024 - qlo):(1024 - qlo) + QTOK]
        gtok = b * S + tok_idx              # global token ids, this batch
        xT_c = xf[gtok].T.astype(ml_dtypes.bfloat16)   # [NS, KTOK]
        # -> [tch, half, kkh, p, f]
        x_all = np.ascontiguousarray(
            xT_c.reshape(4, NSK // 4, 128, TCK, FD).transpose(3, 0, 1, 2, 4))

        # fused weight col-blocks in device order [k, v, q0..q15]
        krows = qkv_w[NH * D + 128 * h:NH * D + 128 * (h + 1)][perm]
        vrows = qkv_w[NG * D + 128 * h:NG * D + 128 * (h + 1)]
        qrows = (qkv_w[2048 * h:2048 * (h + 1)]
                 .reshape(NHL, D, NS)[:, perm] * scale)
        wcols = np.concatenate([krows[None], vrows[None], qrows], 0)  # [18,128,NS]
        # [cb, kk, p(ns), c]
        w_all = np.ascontiguousarray(
            wcols.transpose(2, 0, 1).reshape(NSK, 128, NCB, 128)
            .transpose(2, 0, 1, 3).astype(ml_dtypes.bfloat16))

        bk = qkv_b[NH * D + 128 * h:NH * D + 128 * (h + 1)][perm]
        bv = qkv_b[NG * D + 128 * h:NG * D + 128 * (h + 1)]
        bq = (qkv_b[2048 * h:2048 * (h + 1)].reshape(NHL, D)[:, perm]
              * scale)
        b_all = np.ascontiguousarray(
            np.concatenate([bk[None], bv[None], bq], 0).T.astype(np.float32))

        fc = freqs_cis[b, :, 0, 0]          # [S, 64, 2]
        cos = fc[tok_idx, :, 0].T           # [64, KTOK]
        sin = fc[tok_idx, :, 1].T
        ck = np.concatenate([cos, cos], 0).astype(ml_dtypes.bfloat16)
        sk = np.concatenate([-sin, sin], 0).astype(ml_dtypes.bfloat16)

        # o_w cols for this head-half -> [nb, hd, p(d), f(ns)]
        o_wT = o_w[:, 2048 * h:2048 * (h + 1)].T      # [2048, NS]
        o_wb = np.ascontiguousarray(
            o_wT.reshape(NHL, 128, NSB, FD).transpose(2, 0, 1, 3)
            .astype(ml_dtypes.bfloat16))

        m = dict(x_all=x_all, w_all=w_all, b_all=b_all,
                 ck=np.ascontiguousarray(ck), sk=np.ascontiguousarray(sk),
                 o_wb=o_wb)
        if with_mask:
            mT = attention_mask[b].T        # [key, qpos]
            m["maskT"] = np.ascontiguousarray(
                mT[tok_idx][:, qlo:qlo + QTOK].astype(ml_dtypes.bfloat16))
        in_maps.append(m)
        core_meta.append((h, t, b, sh, gtok))

    res = run_bass_kernel_spmd(nc, in_maps, list(range(8)))
    LAST_EXEC_NS = res.exec_time_ns
    r = res.results

    out = np.empty((NTOK, NS), np.float32)
    for t in range(4):
        b, sh = t // 2, t % 2
        rows = r[t]["outp"] + r[4 + t]["outp"]
        out[b * S + sh * 1024: b * S + sh * 1024 + QTOK] = rows
    out = out.reshape(B, S, NS)

    k_full = np.empty((B, S, NG, 1, D), np.float32)
    v_full = np.empty((B, S, NG, 1, D), np.float32)
    for g in range(2):
        for b in range(2):
            c = g * 4 + 2 * b
            _, _, _, sh, gtok = core_meta[c]
            sidx = gtok - b * S             # seq positions, per-core col order
            k_full[b, sidx, g, 0, :] = r[c]["k_out"][inv].T
            v_full[b, sidx, g, 0, :] = r[c]["v_out"].T
    return out, k_full, v_full
